# revision 32
# baseline (speedup 1.0000x reference)
"""BertGAT Trainium2 kernel: 8-core SPMD bass/Tile implementation.

Model: 2-layer roberta-base BERT feature extraction over 2048 nodes x 16
tokens, then 3 GATConv layers (with edge attributes + self loops) over a
65536-edge random graph.

Sharding: data-parallel over nodes (256 nodes/core) for BERT; vocab-sharded
embedding table + AllReduce for the token embedding gather; GAT edges are
partitioned by destination block (sorted by dst on host), node features
replicated across cores via AllGather, GAT weights replicated.

Self-contained: hardcodes all shapes; host prep is pure index/structure work
(sharding, edge sorting, padding).
"""

import sys

import numpy as np

try:
    import concourse  # noqa: F401
except ImportError:  # pragma: no cover
    sys.path.insert(0, "/opt/trn_rl_repo")

import concourse.bass as bass
import concourse.mybir as mybir
import concourse.tile as tile
from concourse import bacc
from concourse.bass_utils import run_bass_kernel_spmd
from concourse.masks import make_identity

# ---- model dims (hardcoded from the problem spec) ----
V, L, D, NH, FF, NL = 50265, 16, 768, 12, 3072, 2
N, E, NET = 2048, 65536, 6
HC, GH = 64, 4
HD = D // NH  # 64

NCORES = 8
P = 128
NP = N // NCORES          # 256 nodes per core
T = NP * L                # 4096 tokens per core
KS_D = D // P             # 6
KS_FF = FF // P           # 24
V8 = (V + NCORES - 1) // NCORES  # 6284 vocab rows per core
NEG = -1.0e9

F32 = mybir.dt.float32
BF16 = mybir.dt.bfloat16
I32 = mybir.dt.int32
AF = mybir.ActivationFunctionType
OP = mybir.AluOpType

# matmul dtype for the big BERT matmuls
MM_DT = BF16

# dev bisect switch: which BERT sub-phases to emit (e.g. "1", "12", "123")
BERT_PARTS = "1234"
# dev bisect: attention internals depth: 1=S mms, 2=+softmax, 3=full
B2_LEVEL = 3

# GAT per-layer dims: (fin, H, C)
GAT_DIMS = [(D, GH, HC), (GH * HC, GH, HC), (GH * HC, 1, 1)]


# ======================================================================
# host-side prep (pure index/structure work)
# ======================================================================

def _host_prep(x, edge_index, edge_attr):
    x = np.asarray(x, np.int32)
    src = np.asarray(edge_index[0], np.int64)
    dst = np.asarray(edge_index[1], np.int64)
    attr = np.asarray(edge_attr[:, 0], np.int64)

    # per (core, half-block) edge lists, sorted by dst
    order = np.argsort(dst, kind="stable")
    src_s, dst_s, attr_s = src[order], dst[order], attr[order]
    blk_of = dst_s // P  # 16 blocks of 128 dst nodes
    counts = np.bincount(blk_of, minlength=16)
    ep_blk = int(max(1, int(np.max(counts) + P - 1) // P))  # tiles per block

    EP = 2 * ep_blk * P  # padded edges per core
    e_src = np.zeros((NCORES, EP, 1), np.int32)
    e_dst = np.zeros((NCORES, EP, 1), np.int32)
    e_attr = np.full((NCORES, EP, 1), NET, np.int32)  # pad -> row NET (=-1e9)
    e_rel = np.zeros((NCORES, EP, 1), np.float32)
    e_valid = np.zeros((NCORES, EP, 1), np.float32)

    starts = np.concatenate([[0], np.cumsum(counts)])
    for c in range(NCORES):
        for b in range(2):
            g = 2 * c + b
            s, e = starts[g], starts[g + 1]
            n = e - s
            o = b * ep_blk * P
            e_src[c, o:o + n, 0] = src_s[s:e]
            e_dst[c, o:o + n, 0] = dst_s[s:e]
            e_attr[c, o:o + n, 0] = attr_s[s:e]
            e_rel[c, o:o + n, 0] = (dst_s[s:e] - g * P).astype(np.float32)
            e_valid[c, o:o + n, 0] = 1.0

    # Vocab-sharded embedding exchange: owner-grouped AllToAll.
    # Owner r sends, for each requester c, the embedding rows of c's tokens
    # that live in r's vocab slice. S = padded slot count per (owner, req).
    toks = [x[c * NP:(c + 1) * NP].reshape(T) for c in range(NCORES)]
    owner = [t // V8 for t in toks]
    max_n = max(int((owner[c] == r).sum())
                for c in range(NCORES) for r in range(NCORES))
    S = ((max_n + P - 1) // P) * P
    gidx = np.full((NCORES, NCORES * S, 1), 1 << 30, np.int32)
    pidx = np.zeros((NCORES, T, 1), np.int32)
    for c in range(NCORES):
        for r in range(NCORES):
            pos = np.nonzero(owner[c] == r)[0]
            gidx[r, c * S:c * S + len(pos), 0] = toks[c][pos] - r * V8
            pidx[c, pos, 0] = r * S + np.arange(len(pos), dtype=np.int32)

    # own-node global indices (for gather-back of own block rows)
    self_idx = np.zeros((NCORES, NP, 1), np.int32)
    for c in range(NCORES):
        self_idx[c, :, 0] = np.arange(c * NP, (c + 1) * NP, dtype=np.int32)

    # block-diag attention mask [128,128]: 0 within 16-token blocks, -1e9 off
    ii = np.arange(P)
    mask = np.where((ii[:, None] // L) == (ii[None, :] // L), 0.0, NEG)
    mask = mask.astype(np.float32)

    return dict(ep_blk=ep_blk, e_src=e_src, e_dst=e_dst, e_attr=e_attr,
                e_rel=e_rel, e_valid=e_valid, S=S, gidx=gidx, pidx=pidx,
                self_idx=self_idx, mask=mask)


def _shard_params(params, prep):
    """Build per-core in_maps (numpy only)."""
    pr = {k: np.asarray(v, np.float32) for k, v in params.items()
          if k not in ("layers", "gnn")}
    layers = [{k: np.asarray(v, np.float32) for k, v in lp.items()}
              for lp in params["layers"]]
    gnn = [{k: np.asarray(v, np.float32) for k, v in gp.items()}
           for gp in params["gnn"]]

    emb_pad = np.zeros((NCORES * V8, D), np.float32)
    emb_pad[:V] = pr["tok_emb"]

    in_maps = []
    for c in range(NCORES):
        m = {
            "gidx": prep["gidx"][c],
            "pidx": prep["pidx"][c],
            "emb_slice": np.ascontiguousarray(emb_pad[c * V8:(c + 1) * V8]),
            "pos_emb": pr["pos_emb"],
            "emb_g": pr["emb_g"].reshape(1, D),
            "emb_b": pr["emb_b"].reshape(1, D),
            "attn_mask": prep["mask"],
            "edge_emb": pr["edge_emb"],
            "e_src": prep["e_src"][c],
            "e_dst": prep["e_dst"][c],
            "e_attr": prep["e_attr"][c],
            "e_rel": prep["e_rel"][c],
            "e_valid": prep["e_valid"][c],
            "self_idx": prep["self_idx"][c],
        }
        for l, lp in enumerate(layers):
            for k in ("Wq", "Wk", "Wv", "Wo", "W1", "W2"):
                m[f"{k}_{l}"] = lp[k]
            for k in ("bq", "bk", "bv", "bo", "b1", "b2",
                      "ln1_g", "ln1_b", "ln2_g", "ln2_b"):
                m[f"{k}_{l}"] = lp[k].reshape(1, -1)
        for g, gp in enumerate(gnn):
            m[f"Wg_{g}"] = gp["W"]
            m[f"Weg_{g}"] = gp["We"]
            for k in ("att_src", "att_dst", "att_edge", "bias"):
                m[f"{k}_{g}"] = gp[k].reshape(1, -1)
        in_maps.append(m)
    return in_maps


# ======================================================================
# device program
# ======================================================================

class Ctx:
    """Holds nc + shared tiles/constants while building the program."""
    pass


def _declare_io(nc, ep_blk, S):
    io = {}
    DT = lambda n, s, d=F32, k="ExternalInput": nc.dram_tensor(n, s, d, kind=k)
    io["gidx"] = DT("gidx", [NCORES * S, 1], I32)
    io["pidx"] = DT("pidx", [T, 1], I32)
    io["emb_slice"] = DT("emb_slice", [V8, D])
    io["pos_emb"] = DT("pos_emb", [L, D])
    io["emb_g"] = DT("emb_g", [1, D])
    io["emb_b"] = DT("emb_b", [1, D])
    io["attn_mask"] = DT("attn_mask", [P, P])
    io["edge_emb"] = DT("edge_emb", [NET, D])
    EP = 2 * ep_blk * P
    io["e_src"] = DT("e_src", [EP, 1], I32)
    io["e_dst"] = DT("e_dst", [EP, 1], I32)
    io["e_attr"] = DT("e_attr", [EP, 1], I32)
    io["e_rel"] = DT("e_rel", [EP, 1])
    io["e_valid"] = DT("e_valid", [EP, 1])
    io["self_idx"] = DT("self_idx", [NP, 1], I32)
    for l in range(NL):
        for k in ("Wq", "Wk", "Wv", "Wo"):
            io[f"{k}_{l}"] = DT(f"{k}_{l}", [D, D])
        io[f"W1_{l}"] = DT(f"W1_{l}", [D, FF])
        io[f"W2_{l}"] = DT(f"W2_{l}", [FF, D])
        for k in ("bq", "bk", "bv", "bo", "b2", "ln1_g", "ln1_b",
                  "ln2_g", "ln2_b"):
            io[f"{k}_{l}"] = DT(f"{k}_{l}", [1, D])
        io[f"b1_{l}"] = DT(f"b1_{l}", [1, FF])
    for g, (fin, H, C) in enumerate(GAT_DIMS):
        io[f"Wg_{g}"] = DT(f"Wg_{g}", [fin, H * C])
        io[f"Weg_{g}"] = DT(f"Weg_{g}", [D, H * C])
        for k in ("att_src", "att_dst", "att_edge", "bias"):
            io[f"{k}_{g}"] = DT(f"{k}_{g}", [1, H * C])
    io["out"] = DT("out", [NP, 1], F32, "ExternalOutput")
    return io


def _bcast_row(cx, pool, src_ap, width, dtype=F32, name="bc"):
    """DMA a [1, width] DRAM row and broadcast to [128, width] via gpsimd."""
    nc = cx.nc
    row = pool.tile([1, width], dtype, name=f"{name}_row")
    nc.sync.dma_start(row[:], src_ap)
    out = pool.tile([P, width], dtype, name=f"{name}_full")
    nc.gpsimd.partition_broadcast(out[:], row[:])
    return out


def _layernorm(cx, sb, h, g_bc, b_bc, width=D):
    """In-place-ish LN over free dim: returns new tile [P, width] f32."""
    nc = cx.nc
    mu = sb.tile([P, 1], F32, name="ln_mu")
    nc.vector.reduce_sum(mu[:], h[:], axis=mybir.AxisListType.X)
    nc.vector.tensor_scalar_mul(mu[:], mu[:], 1.0 / width)
    xc = sb.tile([P, width], F32, name="ln_xc")
    nc.vector.tensor_scalar(out=xc[:], in0=h[:], scalar1=mu[:, :1],
                            scalar2=None, op0=OP.subtract)
    sq = sb.tile([P, width], F32, name="ln_sq")
    ssq = sb.tile([P, 1], F32, name="ln_ssq")
    nc.scalar.activation(sq[:], xc[:], AF.Square, accum_out=ssq[:])
    # rstd = 1/sqrt(ssq/width + eps)
    std = sb.tile([P, 1], F32, name="ln_std")
    nc.scalar.activation(std[:], ssq[:], AF.Sqrt, bias=cx.eps_col[:, :1],
                         scale=1.0 / width)
    rstd = sb.tile([P, 1], F32, name="ln_rstd")
    nc.vector.reciprocal(rstd[:], std[:])
    o = sb.tile([P, width], F32, name="ln_out")
    # o = (xc * rstd) * g
    nc.vector.scalar_tensor_tensor(out=o[:], in0=xc[:], scalar=rstd[:, :1],
                                   in1=g_bc[:], op0=OP.mult, op1=OP.mult)
    nc.vector.tensor_add(o[:], o[:], b_bc[:])
    return o


def _transpose_to(cx, ps_pool, out_sb_slices, in_tile, ksubs, ident):
    """Transpose [128, ksubs*128] -> ksubs PE transposes into out slices."""
    nc = cx.nc
    for k in range(ksubs):
        pt = ps_pool.tile([P, P], in_tile.dtype, name="tp_ps")
        nc.tensor.transpose(pt[:], in_tile[:, k * P:(k + 1) * P], ident[:])
        nc.vector.tensor_copy(out_sb_slices(k), pt[:])


def _load_w_bf16(cx, wpool, dram_ap, kin, nout, name):
    """Load [kin, nout] f32 weights -> bf16 sbuf [128, kin/128, nout].

    Streams f32 k-slices through a small shared staging slot to avoid
    holding full f32 copies in SBUF.
    """
    nc = cx.nc
    ks = kin // P
    if MM_DT == F32:
        wf = wpool.tile([P, ks, nout], F32, name=f"{name}_f32", tag=f"{name}_f32")
        nc.sync.dma_start(wf[:], dram_ap.rearrange("(o p) n -> p o n", p=P))
        return wf
    wb = wpool.tile([P, ks, nout], MM_DT, name=f"{name}_bf", tag=f"{name}_bf")
    d3 = dram_ap.rearrange("(o p) n -> p o n", p=P)
    for k in range(ks):
        for n0 in range(0, nout, 768):
            nw = min(768, nout - n0)
            stage = wpool.tile([P, 768], F32, name=f"{name}_st{k}_{n0}",
                               tag="wstage", bufs=2)
            nc.sync.dma_start(stage[:, :nw], d3[:, k, n0:n0 + nw])
            nc.vector.tensor_copy(wb[:, k, n0:n0 + nw], stage[:, :nw])
    return wb


# ----------------------------------------------------------------------
# phase: embedding gather + AllReduce + LN -> h_cur
# ----------------------------------------------------------------------

def _phase_embed(cx, tc, io, h_cur, S):
    nc = cx.nc
    with tc.tile_pool(name="emb_sb", bufs=3) as sb, \
         tc.tile_pool(name="emb_dram", bufs=1, space="DRAM") as dr:
        a2a_in = dr.tile([NCORES * S, D], F32)
        a2a_out = dr.tile([NCORES * S, D], F32)
        for t in range(NCORES * S // P):
            et = sb.tile([P, D], F32, name="emb_tile")
            idx = sb.tile([P, 1], I32, name="emb_idx")
            nc.sync.dma_start(idx[:], io["gidx"][t * P:(t + 1) * P, :])
            # pad slots are OOB-skipped; their rows are never referenced
            nc.gpsimd.indirect_dma_start(
                out=et[:], out_offset=None, in_=io["emb_slice"].ap(),
                in_offset=bass.IndirectOffsetOnAxis(ap=idx[:, :1], axis=0),
                bounds_check=V8 - 1, oob_is_err=False)
            nc.sync.dma_start(a2a_in[t * P:(t + 1) * P, :], et[:])
        nc.gpsimd.collective_compute(
            "AllToAll", OP.bypass, replica_groups=[list(range(NCORES))],
            ins=[a2a_in.opt()], outs=[a2a_out.opt()])

        # pos tile [128, D]: pos_emb replicated 8x
        pos = sb.tile([P, D], F32, name="pos_tile", bufs=1)
        for r in range(P // L):
            nc.sync.dma_start(pos[r * L:(r + 1) * L, :], io["pos_emb"].ap())
        g_bc = _bcast_row(cx, sb, io["emb_g"].ap(), D, name="embg")
        b_bc = _bcast_row(cx, sb, io["emb_b"].ap(), D, name="embb")
        for t in range(T // P):
            pidx = sb.tile([P, 1], I32, name="emb_pidx")
            nc.sync.dma_start(pidx[:], io["pidx"][t * P:(t + 1) * P, :])
            ht = sb.tile([P, D], F32, name="emb_h")
            nc.gpsimd.indirect_dma_start(
                out=ht[:], out_offset=None, in_=a2a_out[:],
                in_offset=bass.IndirectOffsetOnAxis(ap=pidx[:, :1], axis=0))
            nc.vector.tensor_add(ht[:], ht[:], pos[:])
            o = _layernorm(cx, sb, ht, g_bc, b_bc)
            nc.sync.dma_start(h_cur[t * P:(t + 1) * P, :], o[:])


# ----------------------------------------------------------------------
# phase: one BERT encoder layer (h_cur -> h_cur)
# ----------------------------------------------------------------------

def _phase_bert_layer(cx, tc, io, l, h_cur, h_mid):
    nc = cx.nc
    CH_T = 1024           # attention chunk tokens
    NCH = T // CH_T       # 4
    TT = CH_T // P        # 8 tiles per chunk

    with tc.tile_pool(name=f"bc_{l}", bufs=1) as cp:
        # per-partition biases for feature-major layouts: [128, 6]
        bq = cp.tile([P, KS_D], F32, name="bq_sb")
        nc.sync.dma_start(bq[:], io[f"bq_{l}"].ap().rearrange("x (o p) -> p (x o)", p=P))
        bk = cp.tile([P, KS_D], F32, name="bk_sb")
        nc.sync.dma_start(bk[:], io[f"bk_{l}"].ap().rearrange("x (o p) -> p (x o)", p=P))
        bv_bc = _bcast_row(cx, cp, io[f"bv_{l}"].ap(), D, name="bv")
        bo_bc = _bcast_row(cx, cp, io[f"bo_{l}"].ap(), D, name="bo")
        g1_bc = _bcast_row(cx, cp, io[f"ln1_g_{l}"].ap(), D, name="g1")
        b1n_bc = _bcast_row(cx, cp, io[f"ln1_b_{l}"].ap(), D, name="b1n")
        g2_bc = _bcast_row(cx, cp, io[f"ln2_g_{l}"].ap(), D, name="g2")
        b2n_bc = _bcast_row(cx, cp, io[f"ln2_b_{l}"].ap(), D, name="b2n")
        mask = cp.tile([P, P], F32, name="mask_sb")
        nc.sync.dma_start(mask[:], io["attn_mask"].ap())
        id_bf = cp.tile([P, P], MM_DT, name="id_bf")
        make_identity(nc, id_bf[:])

        with tc.tile_pool(name=f"bw_{l}", bufs=1) as wp:
            wq = _load_w_bf16(cx, wp, io[f"Wq_{l}"].ap(), D, D, "wq")
            wk = _load_w_bf16(cx, wp, io[f"Wk_{l}"].ap(), D, D, "wk")
            wv = _load_w_bf16(cx, wp, io[f"Wv_{l}"].ap(), D, D, "wv")
            wo = _load_w_bf16(cx, wp, io[f"Wo_{l}"].ap(), D, D, "wo")
            for ch in range(NCH):
                tok0 = ch * CH_T
                with tc.tile_pool(name=f"att_{l}_{ch}", bufs=2) as sb, \
                     tc.tile_pool(name=f"attc_{l}_{ch}", bufs=1) as ck:
                    qt = ck.tile([P, KS_D, CH_T], MM_DT, name="qt_ch")
                    kt = ck.tile([P, KS_D, CH_T], MM_DT, name="kt_ch")
                    vch = ck.tile([P, TT, D], MM_DT, name="v_ch")
                    ot = ck.tile([P, KS_D, CH_T], MM_DT, name="ot_ch")
                    # --- B1: h^T, Q^T, K^T, V ---
                    with tc.tile_pool(name=f"b1p_{l}_{ch}", bufs=2,
                                      space="PSUM") as ps, \
                         tc.tile_pool(name=f"b1h_{l}_{ch}", bufs=1) as hp:
                        hTs = hp.tile([P, TT, KS_D, P], MM_DT, name="hT_ch")
                        for t in range(TT):
                            r0 = tok0 + t * P
                            hf = sb.tile([P, D], F32, name="b1_hf")
                            nc.sync.dma_start(hf[:], h_cur[r0:r0 + P, :])
                            hb = sb.tile([P, D], MM_DT, name="b1_hb")
                            nc.vector.tensor_copy(hb[:], hf[:])
                            _transpose_to(cx, ps,
                                          lambda k, t=t: hTs[:, t, k, :], hb,
                                          KS_D, id_bf)
                            # V tile [128 tok, 768] = hT.T @ Wv
                            for f0 in range(0, D, 512):
                                fw = min(512, D - f0)
                                pv = ps.tile([P, 512], F32, name="b1_pv")
                                for k in range(KS_D):
                                    nc.tensor.matmul(
                                        pv[:, :fw], lhsT=hTs[:, t, k, :],
                                        rhs=wv[:, k, f0:f0 + fw],
                                        start=(k == 0), stop=(k == KS_D - 1))
                                nc.vector.tensor_add(vch[:, t, f0:f0 + fw],
                                                     pv[:, :fw],
                                                     bv_bc[:, f0:f0 + fw])
                        # Q^T/K^T over the whole chunk (free = 512)
                        for dst, w, b in ((qt, wq, bq), (kt, wk, bk)):
                            for m in range(KS_D):
                                for f0 in range(0, CH_T, 512):
                                    pq = ps.tile([P, 512], F32, name="b1_pq")
                                    for k in range(KS_D):
                                        nc.tensor.matmul(
                                            pq[:],
                                            lhsT=w[:, k, m * P:(m + 1) * P],
                                            rhs=hTs[:, f0 // P:f0 // P + 4, k, :],
                                            start=(k == 0),
                                            stop=(k == KS_D - 1))
                                    nc.scalar.activation(
                                        dst[:, m, f0:f0 + 512], pq[:],
                                        AF.Identity, bias=b[:, m:m + 1])
                    # --- B2: attention per tile ---
                    with tc.tile_pool(name=f"b2p_{l}_{ch}", bufs=2,
                                      space="PSUM") as ps:
                        for t in range(TT if "2" in BERT_PARTS else 0):
                            ts_ = slice(t * P, (t + 1) * P)
                            # Even/odd heads use PE row quadrants 0/64, which
                            # run in parallel and must not share a PSUM bank.
                            NHH = NH // 2
                            abs_ = []
                            for par in range(2):
                                sal = ps.tile([P, NHH, P], F32,
                                              name=f"b2_s{par}",
                                              tag=f"b2_s{par}", bufs=1)
                                r = par * HD
                                for kh in range(NHH):
                                    nc.tensor.matmul(
                                        sal[:, kh, :],
                                        lhsT=qt[r:r + HD, kh, ts_],
                                        rhs=kt[r:r + HD, kh, ts_],
                                        start=True, stop=True)
                                sm = sb.tile([P, NHH, P], F32,
                                             name=f"b2_sm{par}",
                                             tag=f"b2_sm{par}")
                                nc.vector.scalar_tensor_tensor(
                                    out=sm[:], in0=sal[:], scalar=0.125,
                                    in1=mask[:, None, :].to_broadcast(
                                        [P, NHH, P]),
                                    op0=OP.mult, op1=OP.add)
                                mx = sb.tile([P, NHH], F32, name=f"b2_mx{par}",
                                             tag=f"b2_mx{par}")
                                nc.vector.reduce_max(mx[:], sm[:],
                                                     axis=mybir.AxisListType.X)
                                nc.vector.tensor_tensor(
                                    out=sm[:], in0=sm[:],
                                    in1=mx[:, :, None].to_broadcast(
                                        [P, NHH, P]),
                                    op=OP.subtract)
                                nc.scalar.activation(sm[:], sm[:], AF.Exp)
                                den = sb.tile([P, NHH], F32,
                                              name=f"b2_den{par}",
                                              tag=f"b2_den{par}")
                                nc.vector.reduce_sum(den[:], sm[:],
                                                     axis=mybir.AxisListType.X)
                                rden = sb.tile([P, NHH], F32,
                                               name=f"b2_rden{par}",
                                               tag=f"b2_rden{par}")
                                nc.vector.reciprocal(rden[:], den[:])
                                ab = sb.tile([P, NHH, P], MM_DT,
                                             name=f"b2_ab{par}",
                                             tag=f"b2_ab{par}")
                                nc.vector.tensor_tensor(
                                    out=ab[:], in0=sm[:],
                                    in1=rden[:, :, None].to_broadcast(
                                        [P, NHH, P]),
                                    op=OP.mult)
                                abs_.append(ab)
                            for k in range(KS_D):
                                po = ps.tile([P, P], F32, name="b2_po")
                                for hh in range(2):
                                    h = 2 * k + hh
                                    at_ps = ps.tile([P, P], MM_DT, name="b2_atp")
                                    nc.tensor.transpose(at_ps[:],
                                                        abs_[hh][:, k, :],
                                                        id_bf[:])
                                    at_sb = sb.tile([P, P], MM_DT, name="b2_ats")
                                    nc.vector.tensor_copy(at_sb[:], at_ps[:])
                                    r = hh * HD
                                    nc.tensor.matmul(
                                        po[r:r + HD, :],
                                        lhsT=vch[:, t, h * HD:(h + 1) * HD],
                                        rhs=at_sb[:], start=True, stop=True)
                                nc.scalar.copy(ot[:, k, ts_], po[:])
                    # --- B3: attn proj + residual + LN1 -> h_mid ---
                    with tc.tile_pool(name=f"b3p_{l}_{ch}", bufs=2,
                                      space="PSUM") as ps:
                        for t in range(TT if "3" in BERT_PARTS else 0):
                            r0 = tok0 + t * P
                            res = sb.tile([P, D], F32, name="b3_res")
                            for f0 in range(0, D, 512):
                                fw = min(512, D - f0)
                                pp = ps.tile([P, 512], F32, name="b3_pp")
                                for k in range(KS_D):
                                    nc.tensor.matmul(
                                        pp[:, :fw],
                                        lhsT=ot[:, k, t * P:(t + 1) * P],
                                        rhs=wo[:, k, f0:f0 + fw],
                                        start=(k == 0), stop=(k == KS_D - 1))
                                nc.vector.tensor_add(res[:, f0:f0 + fw],
                                                     pp[:, :fw],
                                                     bo_bc[:, f0:f0 + fw])
                            hf = sb.tile([P, D], F32, name="b3_hf")
                            nc.sync.dma_start(hf[:], h_cur[r0:r0 + P, :])
                            nc.vector.tensor_add(res[:], res[:], hf[:])
                            o = _layernorm(cx, sb, res, g1_bc, b1n_bc)
                            nc.sync.dma_start(h_mid[r0:r0 + P, :], o[:])

        # --- B4: FFN + residual + LN2 -> h_cur ---
        if "4" not in BERT_PARTS:
            return
        with tc.tile_pool(name=f"bw2_{l}", bufs=1) as wp:
            w1 = _load_w_bf16(cx, wp, io[f"W1_{l}"].ap(), D, FF, "w1")
            w2 = _load_w_bf16(cx, wp, io[f"W2_{l}"].ap(), FF, D, "w2")
            b1 = cp.tile([P, KS_FF], F32, name="b1ffn")
            nc.sync.dma_start(b1[:], io[f"b1_{l}"].ap().rearrange("x (o p) -> p (x o)", p=P))
            b2_bc = _bcast_row(cx, cp, io[f"b2_{l}"].ap(), D, name="b2ffn")
            FCH = 512
            for ch in range(T // FCH):
                tok0 = ch * FCH
                with tc.tile_pool(name=f"ffn_{l}_{ch}", bufs=2) as sb, \
                     tc.tile_pool(name=f"ffnc_{l}_{ch}", bufs=1) as ck, \
                     tc.tile_pool(name=f"ffnp_{l}_{ch}", bufs=2,
                                  space="PSUM") as ps:
                    hTs = ck.tile([P, 4, KS_D, P], MM_DT, name="f_hT")
                    f1t = ck.tile([P, KS_FF, FCH], MM_DT, name="f_f1t")
                    for t in range(4):
                        r0 = tok0 + t * P
                        hf = sb.tile([P, D], F32, name="f_hf")
                        nc.sync.dma_start(hf[:], h_mid[r0:r0 + P, :])
                        hb = sb.tile([P, D], MM_DT, name="f_hb")
                        nc.vector.tensor_copy(hb[:], hf[:])
                        _transpose_to(cx, ps, lambda k, t=t: hTs[:, t, k, :],
                                      hb, KS_D, id_bf)
                    for m in range(KS_FF):
                        pf = ps.tile([P, FCH], F32, name="f_pf")
                        for k in range(KS_D):
                            nc.tensor.matmul(
                                pf[:], lhsT=w1[:, k, m * P:(m + 1) * P],
                                rhs=hTs[:, :, k, :],
                                start=(k == 0), stop=(k == KS_D - 1))
                        nc.scalar.activation(f1t[:, m, :], pf[:],
                                             AF.Gelu_apprx_tanh,
                                             bias=b1[:, m:m + 1])
                    for t in range(4):
                        r0 = tok0 + t * P
                        res = sb.tile([P, D], F32, name="f_res")
                        for f0 in range(0, D, 512):
                            fw = min(512, D - f0)
                            pp = ps.tile([P, 512], F32, name="f_pp")
                            for k in range(KS_FF):
                                nc.tensor.matmul(
                                    pp[:, :fw],
                                    lhsT=f1t[:, k, t * P:(t + 1) * P],
                                    rhs=w2[:, k, f0:f0 + fw],
                                    start=(k == 0), stop=(k == KS_FF - 1))
                            nc.vector.tensor_add(res[:, f0:f0 + fw], pp[:, :fw],
                                                 b2_bc[:, f0:f0 + fw])
                        hf = sb.tile([P, D], F32, name="f_hf2")
                        nc.sync.dma_start(hf[:], h_mid[r0:r0 + P, :])
                        nc.vector.tensor_add(res[:], res[:], hf[:])
                        o = _layernorm(cx, sb, res, g2_bc, b2n_bc)
                        nc.sync.dma_start(h_cur[r0:r0 + P, :], o[:])


# ----------------------------------------------------------------------
# phase: CLS extract + transpose + AllGather -> AGT0 [8*768, 256]
# ----------------------------------------------------------------------

def _phase_cls(cx, tc, io, h_cur, agt0):
    nc = cx.nc
    with tc.tile_pool(name="cls_sb", bufs=2) as sb, \
         tc.tile_pool(name="cls_ps", bufs=2, space="PSUM") as ps, \
         tc.tile_pool(name="cls_dram", bufs=1, space="DRAM") as dr:
        id_f = sb.tile([P, P], F32, name="id_f32", bufs=1)
        make_identity(nc, id_f[:])
        ftl = sb.tile([P, KS_D, NP], F32, name="featsT", bufs=1)
        cls4 = h_cur.ap().rearrange("(n l) d -> n l d", l=L)
        for b in range(NP // P):
            ct = sb.tile([P, D], F32, name="cls_tile")
            nc.sync.dma_start(ct[:], cls4[b * P:(b + 1) * P, 0, :])
            _transpose_to(cx, ps, lambda k, b=b: ftl[:, k, b * P:(b + 1) * P],
                          ct, KS_D, id_f)
        ag_in = dr.tile([D, NP], F32)
        nc.sync.dma_start(ag_in[:].rearrange("(o p) n -> p o n", p=P), ftl[:])
        nc.gpsimd.collective_compute(
            "AllGather", OP.bypass, replica_groups=[list(range(NCORES))],
            ins=[ag_in.opt()], outs=[agt0.ap().opt()])


# ----------------------------------------------------------------------
# phase: one GAT layer
# ----------------------------------------------------------------------

def _phase_gat_layer(cx, tc, io, g, agt_in, agt_out, ep_blk, h_out_dram):
    """agt_in: Shared dram [NCORES*fin, NP] (r-blocked feats^T).
    agt_out: Shared dram for next layer (or None for g=2 -> h_out_dram)."""
    nc = cx.nc
    fin, H, C = GAT_DIMS[g]
    OC = H * C
    KS = fin // P
    XW = OC + H  # xg row: xh | a_src
    with tc.tile_pool(name=f"g{g}_const", bufs=1) as cp, \
         tc.tile_pool(name=f"g{g}_dram", bufs=1, space="DRAM") as dr:
        xg = dr.tile([N, XW], F32)      # gather table: xh | a_src
        adt = dr.tile([N, H], F32)      # a_dst table
        tabp = dr.tile([8, H], F32)     # attr -> a_e table (+ -1e9 pads)

        asrc_bc = _bcast_row(cx, cp, io[f"att_src_{g}"].ap(), OC, name=f"as{g}")
        adst_bc = _bcast_row(cx, cp, io[f"att_dst_{g}"].ap(), OC, name=f"ad{g}")
        aedg_bc = _bcast_row(cx, cp, io[f"att_edge_{g}"].ap(), OC, name=f"ae{g}")
        bias_bc = _bcast_row(cx, cp, io[f"bias_{g}"].ap(), OC, name=f"bi{g}")
        id_f = cp.tile([P, P], F32, name=f"g{g}_id")
        make_identity(nc, id_f[:])
        iota_i = cp.tile([P, P], I32, name=f"g{g}_iota_i")
        nc.gpsimd.iota(iota_i[:], pattern=[[1, P]], base=0, channel_multiplier=0)
        iota_f = cp.tile([P, P], F32, name=f"g{g}_iota")
        nc.vector.tensor_copy(iota_f[:], iota_i[:])

        wg = cp.tile([P, KS, OC], F32, name=f"g{g}_w")
        nc.sync.dma_start(wg[:], io[f"Wg_{g}"].ap().rearrange("(o p) n -> p o n", p=P))
        weg = cp.tile([P, KS_D, OC], F32, name=f"g{g}_we")
        nc.sync.dma_start(weg[:], io[f"Weg_{g}"].ap().rearrange("(o p) n -> p o n", p=P))

        # --- G1a: attr table tabp [8, H] ---
        with tc.tile_pool(name=f"g{g}_t_sb", bufs=2) as sb, \
             tc.tile_pool(name=f"g{g}_t_ps", bufs=2, space="PSUM") as ps:
            ee = sb.tile([NET, D], F32, name="ee_sb")
            nc.sync.dma_start(ee[:], io["edge_emb"].ap())
            eeT = sb.tile([P, KS_D, NET], F32, name="eeT_sb")
            for k in range(KS_D):
                pt = ps.tile([P, NET], F32, name="eeT_ps")
                nc.tensor.transpose(pt[:, :], ee[:, k * P:(k + 1) * P],
                                    id_f[:NET, :NET])
                nc.vector.tensor_copy(eeT[:, k, :], pt[:])
            peh = ps.tile([NET, OC], F32, name="eh_ps")
            for k in range(KS_D):
                nc.tensor.matmul(peh[:], lhsT=eeT[:, k, :], rhs=weg[:, k, :],
                                 start=(k == 0), stop=(k == KS_D - 1))
            ehm = sb.tile([NET, OC], F32, name="ehm_sb")
            nc.vector.tensor_mul(ehm[:], peh[:], aedg_bc[:NET, :])
            tab = sb.tile([8, H], F32, name="tab_sb")
            nc.vector.memset(tab[:], NEG)
            nc.vector.reduce_sum(tab[:NET, :],
                                 ehm[:].rearrange("n (h c) -> n h c", h=H),
                                 axis=mybir.AxisListType.X)
            nc.sync.dma_start(tabp[:], tab[:])

        # --- G1b: xh + a_src + a_dst tables for all N nodes ---
        with tc.tile_pool(name=f"g{g}_x_sb", bufs=3) as sb, \
             tc.tile_pool(name=f"g{g}_x_ps", bufs=2, space="PSUM") as ps:
            for r in range(NCORES):
                for mb in range(NP // P):
                    row0 = r * NP + mb * P
                    ft = sb.tile([P, KS, P], F32, name="g1_ft")
                    nc.sync.dma_start(
                        ft[:], agt_in[r * fin:(r + 1) * fin,
                                      mb * P:(mb + 1) * P].rearrange(
                                          "(o p) n -> p o n", p=P))
                    px = ps.tile([P, OC], F32, name="g1_px")
                    for k in range(KS):
                        nc.tensor.matmul(px[:], lhsT=ft[:, k, :], rhs=wg[:, k, :],
                                         start=(k == 0), stop=(k == KS - 1))
                    xrow = sb.tile([P, XW], F32, name="g1_xrow")
                    nc.vector.tensor_copy(xrow[:, :OC], px[:])
                    tmp = sb.tile([P, OC], F32, name="g1_tmp")
                    nc.vector.tensor_mul(tmp[:], px[:], asrc_bc[:])
                    if C > 1:
                        nc.vector.reduce_sum(
                            xrow[:, OC:XW],
                            tmp[:].rearrange("p (h c) -> p h c", h=H),
                            axis=mybir.AxisListType.X)
                    else:
                        nc.vector.tensor_copy(xrow[:, OC:XW], tmp[:])
                    ad = sb.tile([P, H], F32, name="g1_ad")
                    nc.vector.tensor_mul(tmp[:], px[:], adst_bc[:])
                    if C > 1:
                        nc.vector.reduce_sum(
                            ad[:], tmp[:].rearrange("p (h c) -> p h c", h=H),
                            axis=mybir.AxisListType.X)
                    else:
                        nc.vector.tensor_copy(ad[:], tmp[:])
                    nc.sync.dma_start(xg[row0:row0 + P, :], xrow[:])
                    nc.sync.dma_start(adt[row0:row0 + P, :], ad[:])

        # --- G2: edge aggregation per local dst block ---
        RW = OC + H + 1 + H  # msg | p | valid | ae_masked
        houtT = cp.tile([P, max(OC // P, 1), NP], F32, name=f"g{g}_houtT") \
            if g < 2 else None
        with tc.tile_pool(name=f"g{g}_e_sb", bufs=3) as sb, \
             tc.tile_pool(name=f"g{g}_e_ps", bufs=2, space="PSUM") as ps:
            for blk in range(2):
                acc = ps.tile([P, RW], F32, name="g2_acc")
                for t in range(ep_blk):
                    e0 = (blk * ep_blk + t) * P
                    isrc = sb.tile([P, 1], I32, name="g2_isrc")
                    nc.sync.dma_start(isrc[:], io["e_src"][e0:e0 + P, :])
                    idst = sb.tile([P, 1], I32, name="g2_idst")
                    nc.sync.dma_start(idst[:], io["e_dst"][e0:e0 + P, :])
                    iattr = sb.tile([P, 1], I32, name="g2_iattr")
                    nc.sync.dma_start(iattr[:], io["e_attr"][e0:e0 + P, :])
                    rel = sb.tile([P, 1], F32, name="g2_rel")
                    nc.sync.dma_start(rel[:], io["e_rel"][e0:e0 + P, :])
                    valid = sb.tile([P, 1], F32, name="g2_valid")
                    nc.sync.dma_start(valid[:], io["e_valid"][e0:e0 + P, :])

                    xh_g = sb.tile([P, XW], F32, name="g2_xh")
                    nc.gpsimd.indirect_dma_start(
                        out=xh_g[:], out_offset=None, in_=xg[:],
                        in_offset=bass.IndirectOffsetOnAxis(ap=isrc[:, :1], axis=0))
                    ad_g = sb.tile([P, H], F32, name="g2_ad")
                    nc.gpsimd.indirect_dma_start(
                        out=ad_g[:], out_offset=None, in_=adt[:],
                        in_offset=bass.IndirectOffsetOnAxis(ap=idst[:, :1], axis=0))
                    ae_g = sb.tile([P, H], F32, name="g2_ae")
                    nc.gpsimd.indirect_dma_start(
                        out=ae_g[:], out_offset=None, in_=tabp[:],
                        in_offset=bass.IndirectOffsetOnAxis(ap=iattr[:, :1], axis=0))

                    al = sb.tile([P, H], F32, name="g2_al")
                    nc.vector.tensor_add(al[:], xh_g[:, OC:XW], ad_g[:])
                    nc.vector.tensor_add(al[:], al[:], ae_g[:])
                    nc.scalar.activation(al[:], al[:], AF.Lrelu, alpha=0.2)
                    rhs = sb.tile([P, RW], F32, name="g2_rhs")
                    nc.scalar.activation(rhs[:, OC:OC + H], al[:], AF.Exp)
                    nc.vector.tensor_copy(rhs[:, OC + H:OC + H + 1], valid[:])
                    nc.vector.tensor_scalar_mul(rhs[:, OC + H + 1:RW], ae_g[:],
                                                valid[:, :1])
                    for h in range(H):
                        nc.vector.tensor_scalar_mul(
                            rhs[:, h * C:(h + 1) * C], xh_g[:, h * C:(h + 1) * C],
                            rhs[:, OC + h:OC + h + 1])
                    oh = sb.tile([P, P], F32, name="g2_oh")
                    nc.vector.tensor_scalar(out=oh[:], in0=iota_f[:],
                                            scalar1=rel[:, :1], scalar2=None,
                                            op0=OP.is_equal)
                    nc.tensor.matmul(acc[:], lhsT=oh[:], rhs=rhs[:],
                                     start=(t == 0), stop=(t == ep_blk - 1))

                # --- finalize block: self-loops + normalize ---
                sidx = sb.tile([P, 1], I32, name="g2_sidx")
                nc.sync.dma_start(sidx[:], io["self_idx"][blk * P:(blk + 1) * P, :])
                xh_b = sb.tile([P, XW], F32, name="g2_xhb")
                nc.gpsimd.indirect_dma_start(
                    out=xh_b[:], out_offset=None, in_=xg[:],
                    in_offset=bass.IndirectOffsetOnAxis(ap=sidx[:, :1], axis=0))
                ad_b = sb.tile([P, H], F32, name="g2_adb")
                nc.gpsimd.indirect_dma_start(
                    out=ad_b[:], out_offset=None, in_=adt[:],
                    in_offset=bass.IndirectOffsetOnAxis(ap=sidx[:, :1], axis=0))

                degc = sb.tile([P, 1], F32, name="g2_deg")
                nc.vector.tensor_scalar_max(degc[:], acc[:, OC + H:OC + H + 1], 1.0)
                rdeg = sb.tile([P, 1], F32, name="g2_rdeg")
                nc.vector.reciprocal(rdeg[:], degc[:])
                al = sb.tile([P, H], F32, name="g2_all")
                nc.vector.tensor_scalar_mul(al[:], acc[:, OC + H + 1:RW], rdeg[:, :1])
                nc.vector.tensor_add(al[:], al[:], xh_b[:, OC:XW])
                nc.vector.tensor_add(al[:], al[:], ad_b[:])
                nc.scalar.activation(al[:], al[:], AF.Lrelu, alpha=0.2)
                pl = sb.tile([P, H], F32, name="g2_pl")
                nc.scalar.activation(pl[:], al[:], AF.Exp)
                num = sb.tile([P, OC], F32, name="g2_num")
                for h in range(H):
                    nc.vector.scalar_tensor_tensor(
                        out=num[:, h * C:(h + 1) * C],
                        in0=xh_b[:, h * C:(h + 1) * C], scalar=pl[:, h:h + 1],
                        in1=acc[:, h * C:(h + 1) * C], op0=OP.mult, op1=OP.add)
                den = sb.tile([P, H], F32, name="g2_den")
                nc.vector.tensor_add(den[:], acc[:, OC:OC + H], pl[:])
                rden = sb.tile([P, H], F32, name="g2_rden")
                nc.vector.reciprocal(rden[:], den[:])
                out_b = sb.tile([P, OC], F32, name="g2_out")
                for h in range(H):
                    nc.vector.tensor_scalar_mul(out_b[:, h * C:(h + 1) * C],
                                                num[:, h * C:(h + 1) * C],
                                                rden[:, h:h + 1])
                nc.vector.tensor_add(out_b[:], out_b[:], bias_bc[:, :OC])
                if g < 2:
                    nc.scalar.activation(out_b[:], out_b[:], AF.Relu)
                    for k in range(OC // P):
                        pt = ps.tile([P, P], F32, name="g2_tp")
                        nc.tensor.transpose(pt[:], out_b[:, k * P:(k + 1) * P],
                                            id_f[:])
                        nc.vector.tensor_copy(houtT[:, k, blk * P:(blk + 1) * P],
                                              pt[:])
                else:
                    nc.sync.dma_start(h_out_dram[blk * P:(blk + 1) * P, :],
                                      out_b[:])
            if g < 2:
                ag_in = dr.tile([OC, NP], F32, name="g2_agin")
                nc.sync.dma_start(ag_in[:].rearrange("(o p) n -> p o n", p=P),
                                  houtT[:])
                nc.gpsimd.collective_compute(
                    "AllGather", OP.bypass,
                    replica_groups=[list(range(NCORES))],
                    ins=[ag_in.opt()], outs=[agt_out.ap().opt()])


# ----------------------------------------------------------------------
# full program
# ----------------------------------------------------------------------

def build_program(ep_blk, S, n_bert_layers=NL, n_gat_layers=3, debug=None):
    """debug: None | 'h0'/'bert' (dump h_cur) | 'feats' (dump agt0) |
    'gat0'/'gat1' (dump agt1/agt2)."""
    nc = bacc.Bacc("TRN2", target_bir_lowering=False, debug=False,
                   num_devices=NCORES)
    io = _declare_io(nc, ep_blk, S)
    cx = Ctx()
    cx.nc = nc

    h_cur = nc.dram_tensor("h_cur", [T, D], F32, kind="Internal")
    h_mid = nc.dram_tensor("h_mid", [T, D], F32, kind="Internal")
    agt0 = nc.dram_tensor("agt0", [NCORES * D, NP], F32, kind="Internal",
                          addr_space="Shared")
    agt1 = nc.dram_tensor("agt1", [NCORES * GH * HC, NP], F32, kind="Internal",
                          addr_space="Shared")
    agt2 = nc.dram_tensor("agt2", [NCORES * GH * HC, NP], F32, kind="Internal",
                          addr_space="Shared")
    dbg = None
    if debug in ("h0", "bert"):
        dbg = nc.dram_tensor("dbg", [T, D], F32, kind="ExternalOutput")
    elif debug == "feats":
        dbg = nc.dram_tensor("dbg", [NCORES * D, NP], F32, kind="ExternalOutput")
    elif debug in ("gat0", "gat1"):
        dbg = nc.dram_tensor("dbg", [NCORES * GH * HC, NP], F32,
                             kind="ExternalOutput")

    with tile.TileContext(nc) as tc, \
         tc.tile_pool(name="gconst", bufs=1) as gcp:
        eps_col = gcp.tile([P, 1], F32, name="eps_col")
        nc.vector.memset(eps_col[:], 1e-5)
        cx.eps_col = eps_col
        _phase_embed(cx, tc, io, h_cur, S)
        for l in range(n_bert_layers):
            _phase_bert_layer(cx, tc, io, l, h_cur, h_mid)
        agts = [agt0, agt1, agt2, None]
        if n_gat_layers >= 0:
            _phase_cls(cx, tc, io, h_cur, agt0)
        for g in range(n_gat_layers):
            _phase_gat_layer(cx, tc, io, g, agts[g], agts[g + 1], ep_blk,
                             io["out"] if g == 2 else None)
        if debug in ("h0", "bert"):
            nc.sync.dma_start(dbg.ap(), h_cur.ap())
        elif debug == "feats":
            nc.sync.dma_start(dbg.ap(), agt0.ap())
        elif debug == "gat0":
            nc.sync.dma_start(dbg.ap(), agt1.ap())
        elif debug == "gat1":
            nc.sync.dma_start(dbg.ap(), agt2.ap())
    nc.compile()
    return nc


_CACHE = {}

# test-harness knobs: set TRACE=True before calling kernel() to capture an
# NTFF profile; the BassKernelResults lands in LAST_RESULT.
TRACE = False
LAST_RESULT = None


def _get_program(ep_blk, S):
    key = (ep_blk, S)
    if key not in _CACHE:
        _CACHE[key] = build_program(ep_blk, S)
    return _CACHE[key]


def kernel(x, edge_index, edge_attr, params):
    global LAST_RESULT
    prep = _host_prep(x, edge_index, edge_attr)
    in_maps = _shard_params(params, prep)
    nc = _get_program(prep["ep_blk"], prep["S"])
    res = run_bass_kernel_spmd(nc, in_maps, core_ids=list(range(NCORES)),
                               trace=TRACE)
    LAST_RESULT = res
    out = np.concatenate([res.results[c]["out"] for c in range(NCORES)], axis=0)
    return out.astype(np.float32)
